# revision 7
# baseline (speedup 1.0000x reference)
"""Covariance pooling kernel for Trainium2 (8 NeuronCores, data-parallel over batch).

y[b] = (1/M) * (x[b] - mean(x[b])) @ (x[b] - mean(x[b]))^T  with x[b] [C=128, M=4096].

Strategy per core (8 batches/core):
  - stream each batch tile [128, 4096] f32 from HBM (contiguous, full-BW DMA)
  - PE-transpose each 128-wide chunk (out = chunk.T via identity matmul) -> PSUM
  - copy transposed chunk PSUM->SBUF (alternating DVE/ACT), with a constant
    ones column appended so the Gram matmul also accumulates the row-sum s
  - accumulate G = sum_k Xk^T.T @ [Xk^T | 1] into PSUM ([C, C+1] = [G | s])
  - epilogue: y = (G - s s^T / M) / M via a rank-1 PSUM-accumulated correction

Sync-wait discipline: the TPB LDWEIGHTS slot holds a single wait command, so
every PE instruction is arranged to depend on at most one foreign semaphore
(extra deps are covered transitively by earlier same-engine waits).
"""

import numpy as np

import concourse.bass as bass
import concourse.tile as tile
from concourse import bacc, mybir
from concourse.bass_utils import run_bass_kernel_spmd
from concourse.masks import make_identity

N_CORES = 8
B_FULL = 64
B_CORE = B_FULL // N_CORES  # 8 batches per core
C = 128
M = 4096  # 64*64 spatial
CHUNKS = M // 128  # 32
F32 = mybir.dt.float32
COPY = mybir.ActivationFunctionType.Copy

_CACHE: dict = {}


def _build_program() -> bass.Bass:
    nc = bacc.Bacc()
    x = nc.declare_dram_parameter("x", [B_CORE, C, M], F32, isOutput=False)
    y = nc.declare_dram_parameter("y", [B_CORE, C, C], F32, isOutput=True)

    NSLOT = 8  # transposed-chunk slots in flight

    with tile.TileContext(nc) as tc:
        with (
            tc.tile_pool(name="singles", bufs=1) as singles,
            tc.tile_pool(name="xin", bufs=3) as xin_pool,
            tc.tile_pool(name="yout", bufs=2) as yout_pool,
            tc.tile_pool(name="small", bufs=2) as small_pool,
            tc.tile_pool(name="tp", bufs=3, space="PSUM") as tp_pool,
            tc.tile_pool(name="gram", bufs=2, space="PSUM") as gram_pool,
            tc.tile_pool(name="srow", bufs=2, space="PSUM") as srow_pool,
        ):
            identity = singles.tile([128, 128], F32)
            make_identity(nc, identity)

            # Transposed-chunk ring buffer; col 128 holds the constant 1.0
            # column that makes the Gram matmul also produce the row-sums.
            xt = singles.tile([128, NSLOT, 132], F32)
            nc.vector.memset(xt[:, :, 128:129], 1.0)

            # PE warm-up: absorbs the wait on identity (GpSimd) so the first
            # real transpose only waits on its input DMA (one wait slot).
            warm = tp_pool.tile([128, 128], F32, tag="tp")
            nc.tensor.transpose(warm, identity, identity)

            for b in range(B_CORE):
                x_tile = xin_pool.tile([128, M], F32)
                nc.sync.dma_start(x_tile, x[b])

                gram = gram_pool.tile([128, 129], F32)
                for k in range(CHUNKS):
                    tp = tp_pool.tile([128, 128], F32, tag="tp")
                    nc.tensor.transpose(tp, x_tile[:, k * 128 : (k + 1) * 128], identity)
                    slot = k % NSLOT
                    dst = xt[:, slot, 0:128]
                    if k % 2 == 0:
                        nc.vector.tensor_copy(dst, tp)
                    else:
                        nc.scalar.activation(dst, tp, COPY)
                    nc.tensor.matmul(
                        gram,
                        xt[:, slot, 0:128],
                        xt[:, slot, 0:129],
                        start=(k == 0),
                        stop=False,
                    )

                # epilogue: y = (G - s s^T / M) / M
                # All gram-PSUM readers stay on DVE so the WAR release of the
                # PSUM slot merges with the next batch's DVE waits.
                s_col = small_pool.tile([128, 1], F32)
                nc.vector.tensor_copy(s_col, gram[:, 128:129])
                s_row_ps = srow_pool.tile([1, 128], F32)
                nc.tensor.transpose(s_row_ps, s_col, identity)
                srow = small_pool.tile([1, 128], F32)
                srow_neg = small_pool.tile([1, 128], F32)
                nc.vector.tensor_copy(srow, s_row_ps)
                nc.vector.tensor_scalar_mul(srow_neg, s_row_ps, -1.0 / M)
                nc.tensor.matmul(gram[:, 0:128], srow, srow_neg, start=False, stop=True)

                y_tile = yout_pool.tile([128, 128], F32)
                nc.vector.tensor_scalar_mul(y_tile, gram[:, 0:128], 1.0 / M)
                nc.sync.dma_start(y[b], y_tile)

    nc.compile()  # bacc passes: split multi-waits into event semaphores etc.
    return nc


def _get_program() -> bass.Bass:
    if "nc" not in _CACHE:
        _CACHE["nc"] = _build_program()
    return _CACHE["nc"]


def _run(x: np.ndarray, **spmd_kwargs):
    x = np.ascontiguousarray(np.asarray(x), dtype=np.float32)
    assert x.shape == (B_FULL, C, 64, 64), x.shape
    xf = x.reshape(B_FULL, C, M)
    shards = np.split(xf, N_CORES, axis=0)
    in_maps = [{"x": s} for s in shards]
    nc = _get_program()
    res = run_bass_kernel_spmd(nc, in_maps, list(range(N_CORES)), **spmd_kwargs)
    out = np.concatenate([res.results[i]["y"] for i in range(N_CORES)], axis=0)
    return out, res


def kernel(x: np.ndarray) -> np.ndarray:
    out, _ = _run(x)
    return out
